# revision 13
# baseline (speedup 1.0000x reference)
"""DiverseBeamSearch step on 8 Trainium2 NeuronCores.

Strategy (data parallel over batch):
  - lprobs [32, 12, 50257] f32 is the only large tensor (~77MB). Shard batch
    across 8 cores (4 batch rows -> 48 beam-rows of 50257 per core).
  - Each core computes, for every beam-row, the top-256 values + vocab
    indices of lprobs[row, :] using the GpSimd `topk` instruction
    (one instruction per 8 rows; input laid out as [128, 3144] f32).
  - Host gathers the tiny candidate sets ([32, 12, 256]) and performs the
    exact sequential 4-group diverse-beam logic (diversity penalty, top-3
    selection with jax.lax.top_k tie-break semantics, PAD masking, overlap
    update). Penalties only lower values and touch at most 9 vocab columns
    per group, so the final per-group top-3 always lies within the per-row
    top-12 unpenalized candidates -- 256 candidates make this exact.

Vocab padding: 50257 = 15*3144 + 3097. Each beam-row occupies 16 SBUF
partitions of 3144 elements: partitions 0..14 cover [0, 47160) contiguously
and partition 15 covers [47113, 50257) (47-element overlap instead of a pad
fill; duplicates are removed on the host).
"""

import os
import numpy as np

VOCAB = 50257
CH = 3144                      # per-partition chunk (padded vocab 50304 / 16)
NV = 16 * CH                   # 50304
LAST_START = VOCAB - CH        # 47113
BSZ = 32
BEAM = 12
N_CORES = 8
BATCH_PER_CORE = BSZ // N_CORES          # 4
ROWS_PER_CORE = BATCH_PER_CORE * BEAM    # 48
TOKENS_PER_CALL = 8
CALLS = ROWS_PER_CORE // TOKENS_PER_CALL  # 6
K = 256

PAD = 1
G = 4
MINI = 3
DIVERSITY_STRENGTH = np.float32(-0.5)
DIVERSITY_DISCOUNT = np.float32(0.5)

_cache = {}
LAST_EXEC_NS = None
LAST_RESULTS = None


def _build_bass(with_memset=False):
    # with_memset: the CoreSim write-tracker can't represent the strided DMA
    # coverage of the input tiles and falsely reports uninitialized reads;
    # sim validation builds pre-fill the tiles (hardware builds don't need
    # it -- every byte is DMA-written before the topk reads it).
    import contextlib
    from concourse import bacc, mybir

    nc = bacc.Bacc()
    lp = nc.declare_dram_parameter(
        "lprobs", [ROWS_PER_CORE, VOCAB], mybir.dt.float32, isOutput=False)
    out = nc.declare_dram_parameter(
        "out", [CALLS * 128, 32], mybir.dt.uint32, isOutput=True)

    with contextlib.ExitStack() as ctx:
        inbufs = [ctx.enter_context(
            nc.sbuf_tensor(f"inbuf{c}", [128, CH], mybir.dt.float32))
            for c in range(CALLS)]
        outbufs = [ctx.enter_context(
            nc.sbuf_tensor(f"outbuf{c}", [128, 32], mybir.dt.uint32))
            for c in range(CALLS)]
        in_sems = [ctx.enter_context(nc.semaphore(f"in{c}"))
                   for c in range(CALLS)]
        topk_sem = ctx.enter_context(nc.semaphore("topk"))
        out_sem = ctx.enter_context(nc.semaphore("outd"))
        block = ctx.enter_context(nc.Block())

        if with_memset:
            memset_sem = ctx.enter_context(nc.semaphore("msets"))

            @block.vector
            def _(vector):
                for c in range(CALLS):
                    vector.memset(inbufs[c][:], -3.0e38).then_inc(
                        memset_sem, 1)
                    vector.memset(outbufs[c][:], 0).then_inc(memset_sem, 1)

        @block.sync
        def _(sync):
            if with_memset:
                sync.wait_ge(memset_sem, 2 * CALLS)
            # all input loads are independent (dedicated buffers): issue
            # everything up-front and let the DMA engines saturate HBM
            for c in range(CALLS):
                r0 = c * TOKENS_PER_CALL
                buf = inbufs[c]
                # partition q=0 of each token group holds the final
                # (overlapping) vocab chunk [47113, 50257); q=1..15 hold
                # [0, 47160) contiguously
                srcB = lp[r0:r0 + 8, LAST_START:VOCAB]
                sync.dma_start(out=buf[0:128:16, :], in_=srcB).then_inc(
                    in_sems[c], 16)
                for t in range(TOKENS_PER_CALL):
                    srcA = lp[r0 + t, 0:15 * CH].rearrange(
                        "(q j) -> q j", j=CH)
                    sync.dma_start(out=buf[16 * t + 1:16 * t + 16, :],
                                   in_=srcA).then_inc(in_sems[c], 16)
            for c in range(CALLS):
                sync.wait_ge(topk_sem, c + 1)
                sync.dma_start(out=out[c * 128:(c + 1) * 128, :],
                               in_=outbufs[c][:]).then_inc(out_sem, 16)
            sync.wait_ge(out_sem, 16 * CALLS)

        @block.gpsimd
        def _(gpsimd):
            for c in range(CALLS):
                gpsimd.wait_ge(in_sems[c], 16 * (TOKENS_PER_CALL + 1))
                gpsimd.topk(outbufs[c][:], inbufs[c][:],
                            tokens=TOKENS_PER_CALL, vocab_size=NV,
                            k=K).then_inc(topk_sem, 1)
    return nc


def _get_bass():
    if "nc" not in _cache:
        nc = _build_bass()
        nc.finalize()
        _cache["nc"] = nc
    return _cache["nc"]


def _decode_core_out(out_u32):
    """out_u32: [768, 32] uint32 -> (vals [48, 256] f32, vocab idx [48, 256])."""
    o = np.asarray(out_u32, np.uint32).reshape(CALLS, TOKENS_PER_CALL, 16, 32)
    vals = o[:, :, :, :16].reshape(ROWS_PER_CORE, K).view(np.float32)
    gidx = o[:, :, :, 16:].reshape(ROWS_PER_CORE, K).astype(np.int64)
    # chunk q=0 holds vocab [47113, 50257); chunks 1..15 hold [0, 47160)
    vocab = np.where(gidx < CH, gidx + LAST_START, gidx - CH)
    return vals, vocab


def _host_merge(cand_vals, cand_idx, scores, group_overlap, mask_stop_search,
                original_batch_idxs, step):
    bsz, beam, k = cand_vals.shape
    obi = np.asarray(original_batch_idxs).astype(np.int64)
    go = np.asarray(group_overlap, dtype=np.float32)
    mask3 = np.asarray(mask_stop_search).reshape(bsz, MINI, G)
    step = int(step)
    bias = np.asarray(scores, dtype=np.float32)[:, :, step]

    tokens_G = np.zeros((bsz, MINI, G), np.int64)
    scores_G = np.zeros((bsz, MINI, G), np.float32)
    beams_G = np.zeros((bsz, MINI, G), np.int64)

    for b in range(bsz):
        gob = go[obi[b]]
        for g in range(G):
            vals = []
            flats = []
            for m in range(MINI):
                v = cand_vals[b, g + G * m].astype(np.float32, copy=True)
                ix = cand_idx[b, g + G * m].astype(np.int64)
                order = np.argsort(ix, kind="stable")
                sx = ix[order]
                dup_sorted = np.zeros(k, bool)
                dup_sorted[1:] = sx[1:] == sx[:-1]
                dup = np.zeros(k, bool)
                dup[order] = dup_sorted
                keep = (~dup) & (ix < VOCAB)
                v = v[keep]
                ix = ix[keep]
                if g > 0:
                    # scatter-add order matches the reference: (m2, g2) row-major
                    div = {}
                    for m2 in range(MINI):
                        for g2 in range(g):
                            tok = int(tokens_G[b, m2, g2])
                            pen = np.float32(1.0) + gob[g, g2]
                            div[tok] = np.float32(
                                div.get(tok, np.float32(0.0)) + pen)
                    adj = np.zeros(len(ix), np.float32)
                    for tok, d in div.items():
                        adj[ix == tok] = DIVERSITY_STRENGTH * d
                    v = v + adj
                v = v + bias[b, g + G * m]
                vals.append(v)
                flats.append(m * VOCAB + ix)
            v = np.concatenate(vals)
            f = np.concatenate(flats)
            order = np.lexsort((f, -v))[:3]
            v3 = v[order]
            f3 = f[order]
            beams = f3 // VOCAB
            toks = f3 % VOCAB
            msel = mask3[b, beams, g]
            toks = np.where(msel == 0, PAD, toks)
            scores_G[b, :, g] = v3
            tokens_G[b, :, g] = toks
            beams_G[b, :, g] = beams * G + g

    scores_buf = scores_G.reshape(bsz, MINI * G)
    indices_buf = tokens_G.reshape(bsz, MINI * G).astype(np.int32)
    beams_buf = beams_G.reshape(bsz, MINI * G).astype(np.int32)

    last = tokens_G
    mlast = last != PAD
    ov = (last[:, :, None, :] == last[:, :, :, None]) \
        & mlast[:, :, None, :] & mlast[:, :, :, None]
    overlap = np.sum(ov.astype(np.float32), axis=1)
    new_group_overlap = overlap + DIVERSITY_DISCOUNT * go[obi]
    return scores_buf, indices_buf, beams_buf, new_group_overlap


def _install_ntff_hook():
    """Bridge the missing antenv.axon_hooks module so trace=True works:
    drive NTFF profiling through libaxon_pjrt.so directly (test-time only)."""
    import sys
    import types
    if "antenv.axon_hooks" in sys.modules:
        return
    from trn_agent_boot.trn_boot import _ntff_profile_via_ctypes
    hook = _ntff_profile_via_ctypes("/opt/axon/libaxon_pjrt.so")
    mod = types.ModuleType("antenv.axon_hooks")
    mod.get_axon_ntff_profile_hook = lambda: hook
    sys.modules["antenv.axon_hooks"] = mod
    # the artifact upload needs external storage; keep traces local instead
    from concourse import bass_utils
    bass_utils.upload_artifacts = lambda tmpdir: tmpdir


def kernel(lprobs, scores, group_overlap, mask_stop_search, prev_indices,
           original_batch_idxs, step):
    global LAST_EXEC_NS, LAST_RESULTS
    from concourse.bass_utils import run_bass_kernel_spmd

    lprobs = np.asarray(lprobs, np.float32)
    nc = _get_bass()

    in_maps = []
    for i in range(N_CORES):
        shard = np.ascontiguousarray(
            lprobs[i * BATCH_PER_CORE:(i + 1) * BATCH_PER_CORE]
            .reshape(ROWS_PER_CORE, VOCAB))
        in_maps.append({"lprobs": shard})

    trace = bool(int(os.environ.get("BASS_KERNEL_TRACE", "0")))
    if trace:
        _install_ntff_hook()
    res = run_bass_kernel_spmd(nc, in_maps, core_ids=list(range(N_CORES)),
                               trace=trace)
    LAST_EXEC_NS = res.exec_time_ns
    LAST_RESULTS = res

    cand_vals = np.empty((BSZ, BEAM, K), np.float32)
    cand_idx = np.empty((BSZ, BEAM, K), np.int64)
    for i in range(N_CORES):
        vals, vocab = _decode_core_out(res.results[i]["out"])
        cand_vals[i * BATCH_PER_CORE:(i + 1) * BATCH_PER_CORE] = \
            vals.reshape(BATCH_PER_CORE, BEAM, K)
        cand_idx[i * BATCH_PER_CORE:(i + 1) * BATCH_PER_CORE] = \
            vocab.reshape(BATCH_PER_CORE, BEAM, K)

    return _host_merge(cand_vals, cand_idx, scores, group_overlap,
                       mask_stop_search, original_batch_idxs, step)


# revision 18
# speedup vs baseline: 4.5575x; 4.5575x over previous
"""DiverseBeamSearch step on 8 Trainium2 NeuronCores.

Strategy (data parallel over batch):
  - lprobs [32, 12, 50257] f32 is the only large tensor (~77MB). Shard batch
    across 8 cores (4 batch rows -> 48 beam-rows of 50257 per core).
  - Each core computes per-chunk top-8 values + indices with the DVE `max` /
    `max_index` instructions. Each beam-row is split into 16 chunks of 3142
    elements (stride 3141 -> exact coverage of 50257 with 1-element
    overlaps, no padding). 48 rows x 16 chunks = 768 units, processed as 6
    SBUF slots of [128 partitions, 3142].
  - Host gathers the tiny candidate sets (128 candidates/row) and performs
    the exact sequential 4-group diverse-beam logic (diversity penalty,
    top-3 selection with jax.lax.top_k tie-break semantics, PAD masking,
    overlap update). Penalties only lower values, so the selected top-3 of
    each group must lie above every chunk's 8th-largest value; a
    conservative bound check proves each selection exact, with a (never
    observed, probability ~1e-9) numpy fallback recomputing a batch row
    from the full lprobs if it ever fails.
"""

import os
import numpy as np

VOCAB = 50257
NCHUNK = 16
CH = 3142                      # chunk length
CSTRIDE = 3141                 # chunk stride (1-elem overlap, exact cover)
BSZ = 32
BEAM = 12
N_CORES = 8
BATCH_PER_CORE = BSZ // N_CORES          # 4
ROWS_PER_CORE = BATCH_PER_CORE * BEAM    # 48
SLOTS = 6                                # 768 units / 128 partitions
ROWS_PER_SLOT = 8
K8 = 8
KROW = NCHUNK * K8                       # candidates per row

PAD = 1
G = 4
MINI = 3
DIVERSITY_STRENGTH = np.float32(-0.5)
DIVERSITY_DISCOUNT = np.float32(0.5)

_cache = {}
LAST_EXEC_NS = None
LAST_RESULTS = None
FALLBACKS = 0


def _build_bass():
    import contextlib
    from concourse import bacc, mybir

    nc = bacc.Bacc()
    lp = nc.declare_dram_parameter(
        "lprobs", [ROWS_PER_CORE, VOCAB], mybir.dt.float32, isOutput=False)
    out_vals = nc.declare_dram_parameter(
        "out_vals", [128, SLOTS * K8], mybir.dt.float32, isOutput=True)
    out_idx = nc.declare_dram_parameter(
        "out_idx", [128, SLOTS * K8], mybir.dt.uint16, isOutput=True)

    with contextlib.ExitStack() as ctx:
        inbuf = ctx.enter_context(
            nc.sbuf_tensor("inbuf", [128, SLOTS * CH], mybir.dt.float32))
        vals_sb = ctx.enter_context(
            nc.sbuf_tensor("vals_sb", [128, SLOTS * K8], mybir.dt.float32))
        idx_sb = ctx.enter_context(
            nc.sbuf_tensor("idx_sb", [128, SLOTS * K8], mybir.dt.uint16))
        dma_sems = [ctx.enter_context(nc.semaphore(f"d{u}"))
                    for u in range(SLOTS)]
        msem = ctx.enter_context(nc.semaphore("ms"))
        vsem = ctx.enter_context(nc.semaphore("vs"))
        osem = ctx.enter_context(nc.semaphore("os"))
        block = ctx.enter_context(nc.Block())

        @block.sync
        def _(sync):
            for u in range(SLOTS):
                # slot u: rows [8u, 8u+8), partition p = (r%8)*16 + chunk
                src = bass_ap_slot(lp, u)
                sync.dma_start(out=inbuf[:, u * CH:(u + 1) * CH],
                               in_=src).then_inc(dma_sems[u], 16)
            sync.wait_ge(vsem, SLOTS)
            sync.dma_start(out=out_vals[:, :], in_=vals_sb[:]).then_inc(
                osem, 16)
            sync.dma_start(out=out_idx[:, :], in_=idx_sb[:]).then_inc(
                osem, 16)
            sync.wait_ge(osem, 32)

        @block.vector
        def _(vector):
            for u in range(SLOTS):
                vector.wait_ge(dma_sems[u], 16)
                data = inbuf[:, u * CH:(u + 1) * CH]
                # DVE is deep-pipelined: max_index reads what max wrote, so
                # an explicit same-engine drain point is required
                vector.max(vals_sb[:, u * K8:(u + 1) * K8],
                           data).then_inc(msem, 1)
                vector.wait_ge(msem, u + 1)
                vector.max_index(idx_sb[:, u * K8:(u + 1) * K8],
                                 vals_sb[:, u * K8:(u + 1) * K8],
                                 data).then_inc(vsem, 1)
    return nc


def bass_ap_slot(lp, u):
    """DRAM source AP for slot u: (8 rows) x (16 chunks) x CH, row-major
    over (r, q, j) matching SBUF partitions p = r*16+q."""
    import concourse.bass as bass
    return bass.AP(
        tensor=lp,
        offset=u * ROWS_PER_SLOT * VOCAB,
        ap=[[VOCAB, ROWS_PER_SLOT], [CSTRIDE, NCHUNK], [1, CH]],
    )


def _get_bass():
    if "nc" not in _cache:
        nc = _build_bass()
        nc.finalize()
        _cache["nc"] = nc
    return _cache["nc"]


def _decode_core_out(vals, idx):
    """vals [128, 48] f32, idx [128, 48] u16 ->
    cand_vals [48, 16, 8] f32, cand_vocab [48, 16, 8] i64 per core."""
    vals = np.asarray(vals, np.float32).reshape(128, SLOTS, K8)
    idx = np.asarray(idx).astype(np.int64).reshape(128, SLOTS, K8)
    p = np.arange(128)
    # unit (slot u, partition p) -> row 8u + p//16, chunk q = p%16
    cand_vals = np.empty((ROWS_PER_CORE, NCHUNK, K8), np.float32)
    cand_voc = np.empty((ROWS_PER_CORE, NCHUNK, K8), np.int64)
    rloc = p // 16
    q = p % 16
    for u in range(SLOTS):
        rows = ROWS_PER_SLOT * u + rloc
        cand_vals[rows, q] = vals[:, u]
        cand_voc[rows, q] = q[:, None] * CSTRIDE + idx[:, u]
    return cand_vals, cand_voc


def _host_merge(cand_vals, cand_idx, lprobs, scores, group_overlap,
                mask_stop_search, original_batch_idxs, step):
    """cand_vals/cand_idx: [bsz, beam, NCHUNK, 8] raw top-8 per chunk
    (values descending within each chunk).  lprobs kept for the exact
    fallback path."""
    global FALLBACKS
    bsz = BSZ
    obi = np.asarray(original_batch_idxs).astype(np.int64)
    go = np.asarray(group_overlap, dtype=np.float32)
    mask3 = np.asarray(mask_stop_search).reshape(bsz, MINI, G)
    step = int(step)
    bias = np.asarray(scores, dtype=np.float32)[:, :, step]

    # chunk floors: the 8th (smallest reported) value per chunk bounds every
    # hidden (unreported) element of that chunk from above
    floors = cand_vals[:, :, :, K8 - 1]          # [bsz, beam, NCHUNK]
    row_floor = floors.max(axis=2)               # [bsz, beam]

    flat_vals = cand_vals.reshape(bsz, BEAM, KROW)
    flat_idx = cand_idx.reshape(bsz, BEAM, KROW)

    tokens_G = np.zeros((bsz, MINI, G), np.int64)
    scores_G = np.zeros((bsz, MINI, G), np.float32)
    beams_G = np.zeros((bsz, MINI, G), np.int64)

    for b in range(bsz):
        gob = go[obi[b]]
        use_fallback = False
        for g in range(G):
            # diversity penalty dict (order matches reference scatter-add)
            div = {}
            if g > 0:
                for m2 in range(MINI):
                    for g2 in range(g):
                        tok = int(tokens_G[b, m2, g2])
                        pen = np.float32(1.0) + gob[g, g2]
                        div[tok] = np.float32(
                            div.get(tok, np.float32(0.0)) + pen)

            if not use_fallback:
                vals = []
                flats = []
                hidden_max = -np.inf
                for m in range(MINI):
                    beam_i = g + G * m
                    v = flat_vals[b, beam_i].astype(np.float32, copy=True)
                    ix = flat_idx[b, beam_i]
                    order = np.argsort(ix, kind="stable")
                    sx = ix[order]
                    dup_sorted = np.zeros(KROW, bool)
                    dup_sorted[1:] = sx[1:] == sx[:-1]
                    dup = np.zeros(KROW, bool)
                    dup[order] = dup_sorted
                    keep = ~dup
                    v = v[keep]
                    ix = ix[keep]
                    if div:
                        adj = np.zeros(len(ix), np.float32)
                        for tok, d in div.items():
                            adj[ix == tok] = DIVERSITY_STRENGTH * d
                        v = v + adj
                    v = v + bias[b, beam_i]
                    vals.append(v)
                    flats.append(m * VOCAB + ix)
                    hidden_max = max(hidden_max,
                                     float(row_floor[b, beam_i])
                                     + float(bias[b, beam_i]))
                v = np.concatenate(vals)
                f = np.concatenate(flats)
                order = np.lexsort((f, -v))[:3]
                v3 = v[order]
                f3 = f[order]
                # selection provably exact only if every hidden element is
                # strictly below the 3rd selected value
                if not (hidden_max < float(v3[2])):
                    use_fallback = True

            if use_fallback:
                FALLBACKS += 1
                lp = np.ascontiguousarray(
                    lprobs[b, g::G, :]).astype(np.float32, copy=True)
                for tok, d in div.items():
                    lp[:, tok] = lp[:, tok] + DIVERSITY_STRENGTH * d
                lp = lp + bias[b, g::G][:, None]
                fl = lp.reshape(-1)
                sel = np.lexsort((np.arange(fl.size), -fl))[:3]
                v3 = fl[sel]
                f3 = sel.astype(np.int64)

            beams = f3 // VOCAB
            toks = f3 % VOCAB
            msel = mask3[b, beams, g]
            toks = np.where(msel == 0, PAD, toks)
            scores_G[b, :, g] = v3
            tokens_G[b, :, g] = toks
            beams_G[b, :, g] = beams * G + g

    scores_buf = scores_G.reshape(bsz, MINI * G)
    indices_buf = tokens_G.reshape(bsz, MINI * G).astype(np.int32)
    beams_buf = beams_G.reshape(bsz, MINI * G).astype(np.int32)

    last = tokens_G
    mlast = last != PAD
    ov = (last[:, :, None, :] == last[:, :, :, None]) \
        & mlast[:, :, None, :] & mlast[:, :, :, None]
    overlap = np.sum(ov.astype(np.float32), axis=1)
    new_group_overlap = overlap + DIVERSITY_DISCOUNT * go[obi]
    return scores_buf, indices_buf, beams_buf, new_group_overlap


def _install_ntff_hook():
    """Bridge the missing antenv.axon_hooks module so trace=True works:
    drive NTFF profiling through libaxon_pjrt.so directly (test-time only)."""
    import sys
    import types
    if "antenv.axon_hooks" in sys.modules:
        return
    from trn_agent_boot.trn_boot import _ntff_profile_via_ctypes
    hook = _ntff_profile_via_ctypes("/opt/axon/libaxon_pjrt.so")
    mod = types.ModuleType("antenv.axon_hooks")
    mod.get_axon_ntff_profile_hook = lambda: hook
    sys.modules["antenv.axon_hooks"] = mod
    # the artifact upload needs external storage; keep traces local instead
    from concourse import bass_utils
    bass_utils.upload_artifacts = lambda tmpdir: tmpdir


def kernel(lprobs, scores, group_overlap, mask_stop_search, prev_indices,
           original_batch_idxs, step):
    global LAST_EXEC_NS, LAST_RESULTS
    from concourse.bass_utils import run_bass_kernel_spmd

    lprobs = np.asarray(lprobs, np.float32)
    nc = _get_bass()

    in_maps = []
    for i in range(N_CORES):
        shard = np.ascontiguousarray(
            lprobs[i * BATCH_PER_CORE:(i + 1) * BATCH_PER_CORE]
            .reshape(ROWS_PER_CORE, VOCAB))
        in_maps.append({"lprobs": shard})

    trace = bool(int(os.environ.get("BASS_KERNEL_TRACE", "0")))
    if trace:
        _install_ntff_hook()
    res = run_bass_kernel_spmd(nc, in_maps, core_ids=list(range(N_CORES)),
                               trace=trace)
    LAST_EXEC_NS = res.exec_time_ns
    LAST_RESULTS = res

    cand_vals = np.empty((BSZ, BEAM, NCHUNK, K8), np.float32)
    cand_idx = np.empty((BSZ, BEAM, NCHUNK, K8), np.int64)
    for i in range(N_CORES):
        cv, cx = _decode_core_out(res.results[i]["out_vals"],
                                  res.results[i]["out_idx"])
        cand_vals[i * BATCH_PER_CORE:(i + 1) * BATCH_PER_CORE] = \
            cv.reshape(BATCH_PER_CORE, BEAM, NCHUNK, K8)
        cand_idx[i * BATCH_PER_CORE:(i + 1) * BATCH_PER_CORE] = \
            cx.reshape(BATCH_PER_CORE, BEAM, NCHUNK, K8)

    return _host_merge(cand_vals, cand_idx, lprobs, scores, group_overlap,
                       mask_stop_search, original_batch_idxs, step)
